# revision 25
# baseline (speedup 1.0000x reference)
"""Paged GQA attention (diffusion-LM, bidirectional) on 8 Trainium2 NeuronCores.

Sharding: sequence s -> core s (8 sequences, 8 cores). Each core computes full
attention for one sequence: 32 q heads (8 KV heads x GQA group 4), q_len 64,
context 2048 cached tokens (gathered per block table) + 64 new tokens.

Per-core device kernel (all matmuls bf16, accumulation f32):
  scores^T[tok, g*q] = K^T_chunk.T @ Q^T   (lhsT = K^T chunk [128d,128tok])
  P = exp(scores^T)                         (ScalarE, no max-subtraction:
                                             scores ~ N(0,1), safe in f32)
  [O | denom] += P_chunk^T.T @ [V_chunk | 1] (ones column folds the
                                             partition-dim softmax sum into PE)
  out = O / denom                           (DVE reciprocal + scalar-mul)

K/V stream in group-sized pieces on the sync HWDGE ring in consumption order
(FIFO per ring), with tile-pool slot reuse providing just-in-time backpressure.
Output DMAs ride gpsimd SWDGE so their semaphore waits never stall the input
stream. Host side: block-table gather, head-major transposes, *scale folding
into q, f32->bf16 conversion (halves HBM traffic; abs-max rel err ~6e-3).
"""

import sys
import types

import numpy as np
import ml_dtypes

BF16 = ml_dtypes.bfloat16

# problem constants (hardcoded per spec)
S = 8            # sequences == cores
QL = 64          # active (new) tokens per sequence
NUM_HEADS = 32
HKV = 8          # kv heads
G = 4            # GQA group size
D = 128          # head dim
GQ = G * QL      # 256 q-rows per kv head
MEM_BLK = 32     # tokens per cache block
BLKS = 64        # blocks per sequence
CTX = MEM_BLK * BLKS          # 2048
T = CTX + QL                  # 2112 real tokens
NCH = 17                      # token chunks of 128 (64 tokens padding)
TP = NCH * 128                # 2176 padded tokens
GRPS = (6, 6, 5)              # chunks per exp batch (PSUM-bank sized)
SCALE = 0.08838834764831845

_CACHE = {}


def _install_ntff_hook():
    """bass_utils trace=True under axon needs antenv.axon_hooks; the staged
    antenv package lacks it, so synthesize the module and wire the ctypes
    NTFF hook from trn_agent_boot."""
    import antenv

    if "antenv.axon_hooks" not in sys.modules:
        mod = types.ModuleType("antenv.axon_hooks")
        holder = [None]
        mod.set_axon_ntff_profile_hook = lambda h: holder.__setitem__(0, h)
        mod.get_axon_ntff_profile_hook = lambda: holder[0]
        sys.modules["antenv.axon_hooks"] = mod
        antenv.axon_hooks = mod
    try:
        from trn_agent_boot.trn_boot import _ntff_profile_via_ctypes

        hook = _ntff_profile_via_ctypes("/opt/axon/libaxon_pjrt.so")
        if hook is not None:
            sys.modules["antenv.axon_hooks"].set_axon_ntff_profile_hook(hook)
    except Exception:
        pass


def _build_nc():
    if "nc" in _CACHE:
        return _CACHE["nc"]
    import concourse.bacc as bacc
    import concourse.tile as tile
    from concourse import mybir

    nc = bacc.Bacc("TRN2", target_bir_lowering=False, debug=False, num_devices=S)
    bf = mybir.dt.bfloat16
    f32 = mybir.dt.float32
    # One packed DRAM buffer per (head, group): [K^T group cols | V-aug group
    # chunks | (g==0: Q^T)] so each unit is a single DMA and arrival order is
    # exactly consumption order on one FIFO ring.
    wid = [GQ + GRPS[0] * 128 + GRPS[0] * 129] + [
        gl * 128 + gl * 129 for gl in GRPS[1:]
    ]
    SPL = GQ + 2 * 128  # head-0 piece A: [qt | kt chunks 0-1]
    kv0 = nc.declare_dram_parameter("kv0", [HKV, 128, wid[0]], bf, isOutput=False)
    kv1 = nc.declare_dram_parameter("kv1", [HKV, 128, wid[1]], bf, isOutput=False)
    kv2 = nc.declare_dram_parameter("kv2", [HKV, 128, wid[2]], bf, isOutput=False)
    kv_params = [kv0, kv1, kv2]
    out = nc.declare_dram_parameter("out", [HKV, GQ, D], f32, isOutput=True)

    goff = [0, 6, 12]  # first chunk of each group

    with tile.TileContext(nc) as tc:
        with (
            tc.tile_pool(name="qp", bufs=8) as q_pool,
            tc.tile_pool(name="kv", bufs=12) as kv_pool,
            tc.tile_pool(name="p", bufs=3) as p_pool,
            tc.tile_pool(name="qk", bufs=2, space="PSUM") as qk_pool,
            tc.tile_pool(name="ops", bufs=1, space="PSUM") as o_pool,
            tc.tile_pool(name="osb", bufs=4) as osb_pool,
        ):
            # Each HWDGE DIRECT2D costs ~0.6us of descriptor generation and
            # rings are FIFO, so keep issue counts low and split across both
            # rings: K (+q) on the sync ring, V on the scalar ring (its
            # sequencer is free while the ACT datapath crunches an EXP).
            # Head 0's K is split into group pieces so the first QK batch
            # starts as early as possible; later heads load whole-head.
            # kv_sbs[h, g]: packed tile; col layout [K^T | V-aug | (qt if g0)]
            kv_sbs = {}
            kv0a = kv_pool.tile([128, SPL], bf, tag="kv0a", name="kv0a", bufs=1)
            nc.sync.dma_start(out=kv0a[:], in_=kv_params[0][0][:, 0:SPL])
            kv0b = kv_pool.tile(
                [128, wid[0] - SPL], bf, tag="kv0b", name="kv0b", bufs=1
            )
            nc.sync.dma_start(out=kv0b[:], in_=kv_params[0][0][:, SPL:])
            kv_sbs[0, 0] = (kv0a, kv0b)
            for h in range(HKV):
                for g, gl in enumerate(GRPS):
                    if (h, g) == (0, 0):
                        continue
                    t = kv_pool.tile(
                        [128, wid[g]], bf, tag="kv", name=f"kv_sb{h}_{g}",
                        padded_shape=[128, wid[0]],
                    )
                    nc.sync.dma_start(out=t[:], in_=kv_params[g][h])
                    kv_sbs[h, g] = t

            # Warm the PE HAM clock gate during the DMA ramp: ~3.5us of dummy
            # matmuls with no DMA deps, so real QKs start at 2.4GHz.
            warm_in = osb_pool.tile([128, 512], bf, tag="warm", name="warm_in")
            nc.gpsimd.memset(warm_in[:], 0.0)
            warm_ps = qk_pool.tile([128, 512], f32, tag="qk", name="warm_ps")
            for w in range(5):
                nc.tensor.matmul(
                    warm_ps[:], lhsT=warm_in[:, 0:128], rhs=warm_in[:],
                    start=True, stop=True,
                )

            # Software-pipelined emission over the 24 (head, group) units:
            # QK of unit i+1 is emitted BEFORE PV of unit i so the PE stream
            # never parks behind a PV that waits on the current EXP — keeps
            # ScalarE (the bottleneck) running back-to-back across heads.
            units = [(h, g) for h in range(HKV) for g in range(len(GRPS))]
            o_ps = {}
            p_tiles = {}
            qk_tiles = {}

            def emit_qk(i):
                h, g = units[i]
                gl = GRPS[g]
                qk = qk_pool.tile([128, gl * GQ], f32, tag="qk", name=f"qk{h}_{g}")
                if h == 0:
                    rhs = kv_sbs[0, 0][0][:, 0:GQ]
                else:
                    rhs = kv_sbs[h, 0][:, 0:GQ]

                def kt_ap(cl):
                    if g == 0:
                        if h == 0:
                            a, b = kv_sbs[0, 0]
                            if cl < 2:
                                return a[:, GQ + cl * 128 : GQ + (cl + 1) * 128]
                            return b[:, (cl - 2) * 128 : (cl - 1) * 128]
                        return kv_sbs[h, g][:, GQ + cl * 128 : GQ + (cl + 1) * 128]
                    return kv_sbs[h, g][:, cl * 128 : (cl + 1) * 128]

                for cl in range(gl):
                    nc.tensor.matmul(
                        qk[:, cl * GQ : (cl + 1) * GQ],
                        lhsT=kt_ap(cl),
                        rhs=rhs,
                        start=True,
                        stop=True,
                    )
                qk_tiles[i] = qk

            def emit_exp(i):
                h, g = units[i]
                gl = GRPS[g]
                p_sb = p_pool.tile([128, gl * GQ], bf, tag="p", name=f"p_sb{h}_{g}")
                qk_t = qk_tiles.pop(i)
                if i == 0:
                    # split so ScalarE starts as soon as piece A's chunks land
                    nc.scalar.activation(
                        p_sb[:, 0 : 2 * GQ], qk_t[:, 0 : 2 * GQ],
                        mybir.ActivationFunctionType.Exp,
                    )
                    nc.scalar.activation(
                        p_sb[:, 2 * GQ :], qk_t[:, 2 * GQ :],
                        mybir.ActivationFunctionType.Exp,
                    )
                else:
                    nc.scalar.activation(
                        p_sb[:], qk_t[:], mybir.ActivationFunctionType.Exp
                    )
                p_tiles[i] = p_sb

            def emit_pv(i):
                h, g = units[i]
                gl = GRPS[g]
                if g == 0:
                    o_ps[h] = [
                        o_pool.tile(
                            [128, 129], f32, tag=f"o{half}", name=f"o_ps{h}_{half}"
                        )
                        for half in range(2)
                    ]
                p_sb = p_tiles.pop(i)
                for cl in range(gl):
                    c = goff[g] + cl
                    if (h, g) == (0, 0):
                        b = kv_sbs[0, 0][1]
                        boff = 4 * 128  # kt chunks 2-5 precede va in piece B
                        va_ap = b[:, boff + cl * 129 : boff + (cl + 1) * 129]
                    elif g == 0:
                        t = kv_sbs[h, g]
                        va_ap = t[
                            :, GQ + gl * 128 + cl * 129 : GQ + gl * 128 + (cl + 1) * 129
                        ]
                    else:
                        t = kv_sbs[h, g]
                        va_ap = t[:, gl * 128 + cl * 129 : gl * 128 + (cl + 1) * 129]
                    for half in range(2):
                        nc.tensor.matmul(
                            o_ps[h][half][:],
                            lhsT=p_sb[
                                :, cl * GQ + half * 128 : cl * GQ + (half + 1) * 128
                            ],
                            rhs=va_ap,
                            start=(c == 0),
                            stop=(c == NCH - 1),
                        )
                if g == len(GRPS) - 1:
                    emit_out(h)

            def emit_out(h):
                o_sb = osb_pool.tile([128, 2, D], f32, tag="osb", name=f"o_sb{h}")
                for half in range(2):
                    recip = osb_pool.tile(
                        [128, 1], f32, tag="recip", name=f"recip{h}_{half}"
                    )
                    nc.vector.reciprocal(recip[:], o_ps[h][half][:, 128:129])
                    nc.vector.tensor_scalar_mul(
                        o_sb[:, half, :], o_ps[h][half][:, 0:D], recip[:]
                    )
                # one DMA per head; late heads ride the (by then idle) sync
                # HWDGE ring: ~0.6us latency vs ~2us SWDGE, shorter tail.
                eng = nc.sync if h >= HKV - 2 else nc.gpsimd
                eng.dma_start(
                    out=out[h].rearrange("(a p) d -> p a d", a=2), in_=o_sb[:]
                )

            emit_qk(0)
            emit_qk(1)
            for i in range(len(units)):
                emit_exp(i)
                if i + 2 < len(units):
                    emit_qk(i + 2)
                emit_pv(i)
    nc.compile()
    _CACHE["nc"] = nc
    return nc


def _shard_inputs(q, k, v, k_cache, v_cache, block_tables):
    """Build per-core input maps (host-side gather + layout + bf16).

    Per (head, group) one packed buffer: [K^T group | V-aug group | Q^T (g0)].
    """
    goff = [0, 6, 12]
    in_maps = []
    for s in range(S):
        # Q: [64, 4096] -> [h, d, g*q], scale folded in
        qs = q[s * QL : (s + 1) * QL].reshape(QL, HKV, G, D)
        qt = (qs.transpose(1, 3, 2, 0).reshape(HKV, D, GQ) * SCALE).astype(BF16)

        # K: gather ctx blocks + new tokens -> [T, HKV, D], pad, transpose
        kc = k_cache[block_tables[s]].reshape(CTX, HKV, D)
        kn = k[s * QL : (s + 1) * QL].reshape(QL, HKV, D)
        kf = np.zeros((TP, HKV, D), dtype=np.float32)
        kf[:CTX] = kc
        kf[CTX:T] = kn
        kt = np.ascontiguousarray(kf.transpose(1, 2, 0)).astype(BF16)  # [h, d, tp]

        # V + ones column (zero on padding) -> [h, part, chunk, 129]
        vc = v_cache[block_tables[s]].reshape(CTX, HKV, D)
        vn = v[s * QL : (s + 1) * QL].reshape(QL, HKV, D)
        vf = np.zeros((TP, HKV, D + 1), dtype=np.float32)
        vf[:CTX, :, :D] = vc
        vf[CTX:T, :, :D] = vn
        vf[:T, :, D] = 1.0
        # token t = c*128 + p  ->  va[h, p, c, :]
        va = (
            vf.reshape(NCH, 128, HKV, D + 1)
            .transpose(2, 1, 0, 3)
            .astype(BF16)
        )  # [h, 128, NCH, 129]

        m = {}
        for g, gl in enumerate(GRPS):
            c0 = goff[g]
            parts = []
            if g == 0:
                parts.append(qt)
            parts += [
                kt[:, :, c0 * 128 : (c0 + gl) * 128],                # [h,128,gl*128]
                va[:, :, c0 : c0 + gl, :].reshape(HKV, 128, gl * 129),
            ]
            m[f"kv{g}"] = np.ascontiguousarray(np.concatenate(parts, axis=2))
        in_maps.append(m)
    return in_maps


def _unshard_output(results):
    """Per-core out [HKV, GQ, D] f32 -> full [S*QL, NUM_HEADS*D]."""
    full = np.empty((S * QL, NUM_HEADS * D), dtype=np.float32)
    for s in range(S):
        o = results[s]["out"].reshape(HKV, G, QL, D)
        full[s * QL : (s + 1) * QL] = (
            o.transpose(2, 0, 1, 3).reshape(QL, NUM_HEADS * D)
        )
    return full


def _run(inputs, trace=False):
    from concourse.bass_utils import run_bass_kernel_spmd

    if trace:
        _install_ntff_hook()
    nc = _build_nc()
    in_maps = _shard_inputs(**inputs)
    res = run_bass_kernel_spmd(nc, in_maps, core_ids=list(range(S)), trace=trace)
    return _unshard_output(res.results), res


def kernel(q, k, v, k_cache, v_cache, block_tables):
    out, _ = _run(
        dict(q=q, k=k, v=v, k_cache=k_cache, v_cache=v_cache, block_tables=block_tables)
    )
    return out


# revision 26
# speedup vs baseline: 1.0513x; 1.0513x over previous
"""Paged GQA attention (diffusion-LM, bidirectional) on 8 Trainium2 NeuronCores.

Sharding: sequence s -> core s (8 sequences, 8 cores). Each core computes full
attention for one sequence: 32 q heads (8 KV heads x GQA group 4), q_len 64,
context 2048 cached tokens (gathered per block table) + 64 new tokens.

Per-core device kernel (all matmuls bf16, accumulation f32):
  scores^T[tok, g*q] = K^T_chunk.T @ Q^T   (lhsT = K^T chunk [128d,128tok])
  P = exp(scores^T)                         (ScalarE, no max-subtraction:
                                             scores ~ N(0,1), safe in f32)
  [O | denom] += P_chunk^T.T @ [V_chunk | 1] (ones column folds the
                                             partition-dim softmax sum into PE)
  out = O / denom                           (DVE reciprocal + scalar-mul)

K/V stream in group-sized pieces on the sync HWDGE ring in consumption order
(FIFO per ring), with tile-pool slot reuse providing just-in-time backpressure.
Output DMAs ride gpsimd SWDGE so their semaphore waits never stall the input
stream. Host side: block-table gather, head-major transposes, *scale folding
into q, f32->bf16 conversion (halves HBM traffic; abs-max rel err ~6e-3).
"""

import sys
import types

import numpy as np
import ml_dtypes

BF16 = ml_dtypes.bfloat16

# problem constants (hardcoded per spec)
S = 8            # sequences == cores
QL = 64          # active (new) tokens per sequence
NUM_HEADS = 32
HKV = 8          # kv heads
G = 4            # GQA group size
D = 128          # head dim
GQ = G * QL      # 256 q-rows per kv head
MEM_BLK = 32     # tokens per cache block
BLKS = 64        # blocks per sequence
CTX = MEM_BLK * BLKS          # 2048
T = CTX + QL                  # 2112 real tokens
NCH = 17                      # token chunks of 128 (64 tokens padding)
TP = NCH * 128                # 2176 padded tokens
GRPS = (6, 6, 5)              # chunks per exp batch (PSUM-bank sized)
SCALE = 0.08838834764831845

_CACHE = {}


def _install_ntff_hook():
    """bass_utils trace=True under axon needs antenv.axon_hooks; the staged
    antenv package lacks it, so synthesize the module and wire the ctypes
    NTFF hook from trn_agent_boot."""
    import antenv

    if "antenv.axon_hooks" not in sys.modules:
        mod = types.ModuleType("antenv.axon_hooks")
        holder = [None]
        mod.set_axon_ntff_profile_hook = lambda h: holder.__setitem__(0, h)
        mod.get_axon_ntff_profile_hook = lambda: holder[0]
        sys.modules["antenv.axon_hooks"] = mod
        antenv.axon_hooks = mod
    try:
        from trn_agent_boot.trn_boot import _ntff_profile_via_ctypes

        hook = _ntff_profile_via_ctypes("/opt/axon/libaxon_pjrt.so")
        if hook is not None:
            sys.modules["antenv.axon_hooks"].set_axon_ntff_profile_hook(hook)
    except Exception:
        pass


def _build_nc():
    if "nc" in _CACHE:
        return _CACHE["nc"]
    import concourse.bacc as bacc
    import concourse.tile as tile
    from concourse import mybir

    nc = bacc.Bacc("TRN2", target_bir_lowering=False, debug=False, num_devices=S)
    bf = mybir.dt.bfloat16
    f32 = mybir.dt.float32
    # One packed DRAM buffer per (head, group): [K^T group cols | V-aug group
    # chunks | (g==0: Q^T)] so each unit is a single DMA and arrival order is
    # exactly consumption order on one FIFO ring.
    wid = [GQ + GRPS[0] * 128 + GRPS[0] * 129] + [
        gl * 128 + gl * 129 for gl in GRPS[1:]
    ]
    SPL = GQ + 2 * 128  # head-0 piece A: [qt | kt chunks 0-1]
    kv0 = nc.declare_dram_parameter("kv0", [HKV, 128, wid[0]], bf, isOutput=False)
    kv1 = nc.declare_dram_parameter("kv1", [HKV, 128, wid[1]], bf, isOutput=False)
    kv2 = nc.declare_dram_parameter("kv2", [HKV, 128, wid[2]], bf, isOutput=False)
    kv_params = [kv0, kv1, kv2]
    out = nc.declare_dram_parameter("out", [HKV, GQ, D], f32, isOutput=True)

    goff = [0, 6, 12]  # first chunk of each group

    with tile.TileContext(nc) as tc:
        with (
            tc.tile_pool(name="qp", bufs=8) as q_pool,
            tc.tile_pool(name="kv", bufs=12) as kv_pool,
            tc.tile_pool(name="p", bufs=3) as p_pool,
            tc.tile_pool(name="qk", bufs=2, space="PSUM") as qk_pool,
            tc.tile_pool(name="ops", bufs=1, space="PSUM") as o_pool,
            tc.tile_pool(name="osb", bufs=4) as osb_pool,
        ):
            # Each HWDGE DIRECT2D costs ~0.6us of descriptor generation and
            # rings are FIFO, so keep issue counts low and split across both
            # rings: K (+q) on the sync ring, V on the scalar ring (its
            # sequencer is free while the ACT datapath crunches an EXP).
            # Head 0's K is split into group pieces so the first QK batch
            # starts as early as possible; later heads load whole-head.
            # kv_sbs[h, g]: packed tile; col layout [K^T | V-aug | (qt if g0)]
            kv_sbs = {}
            KTE = GQ + GRPS[0] * 128  # end of K cols in the g0 pack
            kv0a = kv_pool.tile([128, SPL], bf, tag="kv0a", name="kv0a", bufs=1)
            nc.sync.dma_start(out=kv0a[:], in_=kv_params[0][0][:, 0:SPL])
            kv0a2 = kv_pool.tile(
                [128, KTE - SPL], bf, tag="kv0a2", name="kv0a2", bufs=1
            )
            nc.sync.dma_start(out=kv0a2[:], in_=kv_params[0][0][:, SPL:KTE])
            kv0b = kv_pool.tile(
                [128, wid[0] - KTE], bf, tag="kv0b", name="kv0b", bufs=1
            )
            nc.sync.dma_start(out=kv0b[:], in_=kv_params[0][0][:, KTE:])
            kv_sbs[0, 0] = (kv0a, kv0a2, kv0b)
            for h in range(HKV):
                for g, gl in enumerate(GRPS):
                    if (h, g) == (0, 0):
                        continue
                    t = kv_pool.tile(
                        [128, wid[g]], bf, tag="kv", name=f"kv_sb{h}_{g}",
                        padded_shape=[128, wid[0]],
                    )
                    nc.sync.dma_start(out=t[:], in_=kv_params[g][h])
                    kv_sbs[h, g] = t

            # Warm the PE HAM clock gate during the DMA ramp: ~3.5us of dummy
            # matmuls with no DMA deps, so real QKs start at 2.4GHz.
            warm_in = osb_pool.tile([128, 512], bf, tag="warm", name="warm_in")
            nc.gpsimd.memset(warm_in[:], 0.0)
            warm_ps = qk_pool.tile([128, 512], f32, tag="qk", name="warm_ps")
            for w in range(5):
                nc.tensor.matmul(
                    warm_ps[:], lhsT=warm_in[:, 0:128], rhs=warm_in[:],
                    start=True, stop=True,
                )

            # Software-pipelined emission over the 24 (head, group) units:
            # QK of unit i+1 is emitted BEFORE PV of unit i so the PE stream
            # never parks behind a PV that waits on the current EXP — keeps
            # ScalarE (the bottleneck) running back-to-back across heads.
            units = [(h, g) for h in range(HKV) for g in range(len(GRPS))]
            o_ps = {}
            p_tiles = {}
            qk_tiles = {}

            def emit_qk(i):
                h, g = units[i]
                gl = GRPS[g]
                if h == 0:
                    rhs = kv_sbs[0, 0][0][:, 0:GQ]
                else:
                    rhs = kv_sbs[h, 0][:, 0:GQ]

                def kt_ap(cl):
                    if g == 0:
                        if h == 0:
                            a, a2, _ = kv_sbs[0, 0]
                            if cl < 2:
                                return a[:, GQ + cl * 128 : GQ + (cl + 1) * 128]
                            return a2[:, (cl - 2) * 128 : (cl - 1) * 128]
                        return kv_sbs[h, g][:, GQ + cl * 128 : GQ + (cl + 1) * 128]
                    return kv_sbs[h, g][:, cl * 128 : (cl + 1) * 128]

                if i == 0:
                    # two PSUM tiles so the first exp isn't gated on chunks 2-5
                    # (tile deps are tile-granular)
                    qka = qk_pool.tile([128, 2 * GQ], f32, tag="qk", name="qk0a")
                    qkb = qk_pool.tile([128, 4 * GQ], f32, tag="qk", name="qk0b")
                    for cl in range(gl):
                        dst = (
                            qka[:, cl * GQ : (cl + 1) * GQ] if cl < 2
                            else qkb[:, (cl - 2) * GQ : (cl - 1) * GQ]
                        )
                        nc.tensor.matmul(
                            dst, lhsT=kt_ap(cl), rhs=rhs, start=True, stop=True
                        )
                    qk_tiles[i] = (qka, qkb)
                    return
                qk = qk_pool.tile([128, gl * GQ], f32, tag="qk", name=f"qk{h}_{g}")
                for cl in range(gl):
                    nc.tensor.matmul(
                        qk[:, cl * GQ : (cl + 1) * GQ],
                        lhsT=kt_ap(cl),
                        rhs=rhs,
                        start=True,
                        stop=True,
                    )
                qk_tiles[i] = qk

            def emit_exp(i):
                h, g = units[i]
                gl = GRPS[g]
                p_sb = p_pool.tile([128, gl * GQ], bf, tag="p", name=f"p_sb{h}_{g}")
                qk_t = qk_tiles.pop(i)
                if i == 0:
                    qka, qkb = qk_t
                    nc.scalar.activation(
                        p_sb[:, 0 : 2 * GQ], qka[:],
                        mybir.ActivationFunctionType.Exp,
                    )
                    nc.scalar.activation(
                        p_sb[:, 2 * GQ :], qkb[:],
                        mybir.ActivationFunctionType.Exp,
                    )
                else:
                    nc.scalar.activation(
                        p_sb[:], qk_t[:], mybir.ActivationFunctionType.Exp
                    )
                p_tiles[i] = p_sb

            def emit_pv(i):
                h, g = units[i]
                gl = GRPS[g]
                if g == 0:
                    o_ps[h] = [
                        o_pool.tile(
                            [128, 129], f32, tag=f"o{half}", name=f"o_ps{h}_{half}"
                        )
                        for half in range(2)
                    ]
                p_sb = p_tiles.pop(i)
                for cl in range(gl):
                    c = goff[g] + cl
                    if (h, g) == (0, 0):
                        b = kv_sbs[0, 0][2]
                        va_ap = b[:, cl * 129 : (cl + 1) * 129]
                    elif g == 0:
                        t = kv_sbs[h, g]
                        va_ap = t[
                            :, GQ + gl * 128 + cl * 129 : GQ + gl * 128 + (cl + 1) * 129
                        ]
                    else:
                        t = kv_sbs[h, g]
                        va_ap = t[:, gl * 128 + cl * 129 : gl * 128 + (cl + 1) * 129]
                    for half in range(2):
                        nc.tensor.matmul(
                            o_ps[h][half][:],
                            lhsT=p_sb[
                                :, cl * GQ + half * 128 : cl * GQ + (half + 1) * 128
                            ],
                            rhs=va_ap,
                            start=(c == 0),
                            stop=(c == NCH - 1),
                        )
                if g == len(GRPS) - 1:
                    emit_out(h)

            def emit_out(h):
                o_sb = osb_pool.tile([128, 2, D], f32, tag="osb", name=f"o_sb{h}")
                for half in range(2):
                    recip = osb_pool.tile(
                        [128, 1], f32, tag="recip", name=f"recip{h}_{half}"
                    )
                    nc.vector.reciprocal(recip[:], o_ps[h][half][:, 128:129])
                    nc.vector.tensor_scalar_mul(
                        o_sb[:, half, :], o_ps[h][half][:, 0:D], recip[:]
                    )
                # one DMA per head; late heads ride the (by then idle) sync
                # HWDGE ring: ~0.6us latency vs ~2us SWDGE, shorter tail.
                eng = nc.sync if h >= HKV - 2 else nc.gpsimd
                eng.dma_start(
                    out=out[h].rearrange("(a p) d -> p a d", a=2), in_=o_sb[:]
                )

            emit_qk(0)
            emit_qk(1)
            for i in range(len(units)):
                emit_exp(i)
                if i + 2 < len(units):
                    emit_qk(i + 2)
                emit_pv(i)
    nc.compile()
    _CACHE["nc"] = nc
    return nc


def _shard_inputs(q, k, v, k_cache, v_cache, block_tables):
    """Build per-core input maps (host-side gather + layout + bf16).

    Per (head, group) one packed buffer: [K^T group | V-aug group | Q^T (g0)].
    """
    goff = [0, 6, 12]
    in_maps = []
    for s in range(S):
        # Q: [64, 4096] -> [h, d, g*q], scale folded in
        qs = q[s * QL : (s + 1) * QL].reshape(QL, HKV, G, D)
        qt = (qs.transpose(1, 3, 2, 0).reshape(HKV, D, GQ) * SCALE).astype(BF16)

        # K: gather ctx blocks + new tokens -> [T, HKV, D], pad, transpose
        kc = k_cache[block_tables[s]].reshape(CTX, HKV, D)
        kn = k[s * QL : (s + 1) * QL].reshape(QL, HKV, D)
        kf = np.zeros((TP, HKV, D), dtype=np.float32)
        kf[:CTX] = kc
        kf[CTX:T] = kn
        kt = np.ascontiguousarray(kf.transpose(1, 2, 0)).astype(BF16)  # [h, d, tp]

        # V + ones column (zero on padding) -> [h, part, chunk, 129]
        vc = v_cache[block_tables[s]].reshape(CTX, HKV, D)
        vn = v[s * QL : (s + 1) * QL].reshape(QL, HKV, D)
        vf = np.zeros((TP, HKV, D + 1), dtype=np.float32)
        vf[:CTX, :, :D] = vc
        vf[CTX:T, :, :D] = vn
        vf[:T, :, D] = 1.0
        # token t = c*128 + p  ->  va[h, p, c, :]
        va = (
            vf.reshape(NCH, 128, HKV, D + 1)
            .transpose(2, 1, 0, 3)
            .astype(BF16)
        )  # [h, 128, NCH, 129]

        m = {}
        for g, gl in enumerate(GRPS):
            c0 = goff[g]
            parts = []
            if g == 0:
                parts.append(qt)
            parts += [
                kt[:, :, c0 * 128 : (c0 + gl) * 128],                # [h,128,gl*128]
                va[:, :, c0 : c0 + gl, :].reshape(HKV, 128, gl * 129),
            ]
            m[f"kv{g}"] = np.ascontiguousarray(np.concatenate(parts, axis=2))
        in_maps.append(m)
    return in_maps


def _unshard_output(results):
    """Per-core out [HKV, GQ, D] f32 -> full [S*QL, NUM_HEADS*D]."""
    full = np.empty((S * QL, NUM_HEADS * D), dtype=np.float32)
    for s in range(S):
        o = results[s]["out"].reshape(HKV, G, QL, D)
        full[s * QL : (s + 1) * QL] = (
            o.transpose(2, 0, 1, 3).reshape(QL, NUM_HEADS * D)
        )
    return full


def _run(inputs, trace=False):
    from concourse.bass_utils import run_bass_kernel_spmd

    if trace:
        _install_ntff_hook()
    nc = _build_nc()
    in_maps = _shard_inputs(**inputs)
    res = run_bass_kernel_spmd(nc, in_maps, core_ids=list(range(S)), trace=trace)
    return _unshard_output(res.results), res


def kernel(q, k, v, k_cache, v_cache, block_tables):
    out, _ = _run(
        dict(q=q, k=k, v=v, k_cache=k_cache, v_cache=v_cache, block_tables=block_tables)
    )
    return out
